# revision 1
# baseline (speedup 1.0000x reference)
"""Trainium2 Bass kernel for the LSTM cell forecaster.

Same model/distribution as kernel.py (data-parallel over batch, 8 cores,
512 rows/core; hidden units on partitions, batch on free dim). Differences
from v1:

  - fp16 for everything entering the PE (weights, x, h, y_stage) so matmul
    cost is 1.0 cycles/row at any chunk size; PSUM accumulation stays f32.
  - Batch split into NCHAIN independent chunk-chains (default 2); per chunk
    and step: 4 h-matmuls -> fused sigmoid over all 4 gates (f,i,o,g with g
    pre-scaled by 2) -> t1 = sf*c on Pool, t2 = (sg-.5)*si on DVE,
    c = 2*t2 + t1 on DVE -> tanh(c) on ACT -> h = so*tanh_c on DVE (fp16,
    2x mode).
  - Emission order per step: sigma(chunk), tanh(chunk) pairs so the ACT
    queue never head-of-line blocks a ready tanh behind the other chunk's
    big sigmoid.
  - Bias folded into the x-projection via a ones row (K=3); forecast bias
    fc_b folded into the forecast x-weights; fc_b added back on host.
"""

import sys

for _p in ("/opt/trn_rl_repo",):
    if _p not in sys.path:
        sys.path.insert(0, _p)

import numpy as np

import concourse.bass as bass
import concourse.bacc as bacc
import concourse.mybir as mybir
import concourse.tile as tile
from concourse.bass_utils import run_bass_kernel_spmd

B_TOT = 4096
T = 512
IN = 2
H = 128
OUT = 2
FUT = 50
NCORES = 8
B = B_TOT // NCORES  # 512 batch rows per core
NSTEPS = T + FUT - 1

F32 = mybir.dt.float32
F16 = mybir.dt.float16
AF = mybir.ActivationFunctionType
ALU = mybir.AluOpType

VARIANT = {
    "nchain": 4,
    "x_blk": 8,             # steps per x DMA block
    "x_prefetch": 3,        # x blocks resident
    "t1_engine": "vector",  # gpsimd | vector | split (low half Pool, high DVE)
    "x_filler": 0,          # extra redundant x-matmul rounds (PE pstate hold)
    "act_f32": False,       # f32 sigmoid/tanh outputs (ACT internal precision)
    "sigo_split": False,    # sigma(f,i,g) first, sigma(o) separate (off-path)
    "tanh_lag": True,       # emit tanh(chunk) after sigma(chunk+1): at P/4
                            # stagger the next sigma's inputs are ready first
    "layout": "shared",     # shared: one [H,4,B] tile, double-buffered.
                            # padded: per-chunk single-buffered tiles with
                            # each gate plane padded to a full 2KB bank
                            # (nchain=2 only; decouples the chunk chains).
}

# Gate order in PSUM/weights: f, i, g, o (torch order is i, f, g, o).
_TORCH_SLOT = {"i": 0, "f": 1, "g": 2, "o": 3}
_GATES = ("f", "i", "g", "o")


def _build_nc(nsteps=NSTEPS, timing_reps=1, dump_state=False):
    nchain = VARIANT["nchain"]
    xpf = VARIANT["x_prefetch"]
    xblk = VARIANT["x_blk"]
    bounds = [round(i * B / nchain) for i in range(nchain + 1)]
    chunks = [slice(bounds[i], bounds[i + 1]) for i in range(nchain)]

    nc = bacc.Bacc("TRN2", target_bir_lowering=False)

    x_aug = nc.dram_tensor("x_aug", [T // xblk, 3, xblk, B], F16, kind="ExternalInput")
    w_hh = nc.dram_tensor("w_hh", [H, 4, H], F16, kind="ExternalInput")
    w_ih_e = nc.dram_tensor("w_ih_e", [3, 4, H], F16, kind="ExternalInput")
    w_ih_f = nc.dram_tensor("w_ih_f", [3, 4, H], F16, kind="ExternalInput")
    fc_wt = nc.dram_tensor("fc_wt", [H, OUT], F16, kind="ExternalInput")
    ones3 = nc.dram_tensor("ones3", [3, B], F16, kind="ExternalInput")
    y_out = nc.dram_tensor("y_out", [OUT, FUT, B], F16, kind="ExternalOutput")
    if dump_state:
        h_out = nc.dram_tensor("h_out", [H, B], F16, kind="ExternalOutput")
        c_out = nc.dram_tensor("c_out", [H, B], F32, kind="ExternalOutput")
        sig_out = nc.dram_tensor("sig_out", [H, 4, B], F16, kind="ExternalOutput")
        t1_out = nc.dram_tensor("t1_out", [H, B], F32, kind="ExternalOutput")
        t2_out = nc.dram_tensor("t2_out", [H, B], F32, kind="ExternalOutput")

    with tile.TileContext(nc) as tc:
        with (
            tc.tile_pool(name="consts", bufs=1) as consts,
            tc.tile_pool(name="state", bufs=1) as state,
            tc.tile_pool(name="xpool", bufs=xpf) as xpool,
            tc.tile_pool(
                name="psum",
                bufs=1 if VARIANT["layout"] == "padded" else 2,
                space="PSUM",
            ) as psum,
        ):
            w_hh_sb = consts.tile([H, 4, H], F16)
            nc.sync.dma_start(out=w_hh_sb, in_=w_hh[:, :, :])
            w_ih_e_sb = consts.tile([3, 4, H], F16)
            nc.sync.dma_start(out=w_ih_e_sb, in_=w_ih_e[:, :, :])
            w_ih_f_sb = consts.tile([3, 4, H], F16)
            nc.sync.dma_start(out=w_ih_f_sb, in_=w_ih_f[:, :, :])
            fc_wt_sb = consts.tile([H, OUT], F16)
            nc.sync.dma_start(out=fc_wt_sb, in_=fc_wt[:, :])

            SIGDT = F32 if VARIANT["act_f32"] else F16
            h_sb = state.tile([H, B], F16)
            c_sb = state.tile([H, B], F32)
            tc_sb = state.tile([H, B], SIGDT)
            sig_sb = state.tile([H, 4, B], SIGDT)
            t1_sb = state.tile([H, B], F32)
            t2_sb = state.tile([H, B], F32)
            y_stage = state.tile([3, B], F16)

            nc.vector.memset(c_sb, 0.0)
            nc.vector.memset(h_sb, 0.0)
            # Row 2 is the constant ones row (bias trick); rows 0-1 are
            # overwritten by the forecast y copy before any read.
            nc.sync.dma_start(out=y_stage, in_=ones3[:, :])

            x_tiles = {}

            def fetch_x_block(bi):
                if bi < T // xblk:
                    xt = xpool.tile([3, xblk, B], F16, name=f"xb_{bi}", tag="x")
                    nc.sync.dma_start(out=xt, in_=x_aug[bi, :, :, :])
                    x_tiles[bi] = xt

            def x_rhs(t):
                if t < T:
                    return x_tiles[t // xblk][:, t % xblk, :], w_ih_e_sb
                return y_stage, w_ih_f_sb

            def x_matmuls_full(gt, t):
                # Full-batch per gate: matmul start=True resets the whole
                # 2KB PSUM bank it targets, so each accumulation group must
                # own its bank exclusively -> one [H, B] plane (= 1 bank)
                # per gate, written by a single start=True matmul.
                rhs, lhs = x_rhs(t)
                for g in range(4):
                    nc.tensor.matmul(
                        gt[:, g, :],
                        lhsT=lhs[:, g, :],
                        rhs=rhs[:, :],
                        start=True,
                        stop=(t == 0),
                        skip_group_check=True,
                    )

            def h_matmuls_chunk(gt, sl, start=False):
                for g in range(4):
                    nc.tensor.matmul(
                        gt[:, g, sl],
                        lhsT=w_hh_sb[:, g, :],
                        rhs=h_sb[:, sl],
                        start=start,
                        stop=True,
                        skip_group_check=True,
                    )

            t1_mode = VARIANT["t1_engine"]

            def emit_t1(sl):
                if t1_mode == "split":
                    lo = slice(sl.start, (sl.start + sl.stop) // 2)
                    hi = slice((sl.start + sl.stop) // 2, sl.stop)
                    nc.gpsimd.tensor_mul(t1_sb[:, lo], sig_sb[:, 0, lo], c_sb[:, lo])
                    nc.vector.tensor_mul(t1_sb[:, hi], sig_sb[:, 0, hi], c_sb[:, hi])
                elif t1_mode == "gpsimd":
                    nc.gpsimd.tensor_mul(t1_sb[:, sl], sig_sb[:, 0, sl], c_sb[:, sl])
                else:
                    nc.vector.tensor_mul(t1_sb[:, sl], sig_sb[:, 0, sl], c_sb[:, sl])

            def chunk_head(cur, ci, sl, psl=None):
                """sigma -> (t1, t2) -> c for one chunk."""
                if psl is None:
                    psl = sl
                if VARIANT["sigo_split"]:
                    nc.scalar.activation(sig_sb[:, 0:3, sl], cur[:, 0:3, psl], AF.Sigmoid)
                    nc.scalar.activation(sig_sb[:, 3, sl], cur[:, 3, psl], AF.Sigmoid)
                else:
                    nc.scalar.activation(sig_sb[:, 0:4, sl], cur[:, 0:4, psl], AF.Sigmoid)
                emit_t1(sl)
                nc.vector.scalar_tensor_tensor(
                    t2_sb[:, sl],
                    in0=sig_sb[:, 2, sl],
                    scalar=0.5,
                    in1=sig_sb[:, 1, sl],
                    op0=ALU.subtract,
                    op1=ALU.mult,
                )
                nc.vector.scalar_tensor_tensor(
                    c_sb[:, sl],
                    in0=t2_sb[:, sl],
                    scalar=2.0,
                    in1=t1_sb[:, sl],
                    op0=ALU.mult,
                    op1=ALU.add,
                )
            def chunk_tail2(ci, sl):
                """tanh(c) -> h for one chunk."""
                nc.scalar.activation(tc_sb[:, sl], c_sb[:, sl], AF.Tanh)
                nc.vector.tensor_mul(h_sb[:, sl], sig_sb[:, 3, sl], tc_sb[:, sl])

            def chunk_tail(cur, ci, sl, psl=None):
                chunk_head(cur, ci, sl, psl)
                chunk_tail2(ci, sl)

            def y_block(t, cur_gts):
                """Forecast output: y = fc_w @ h into the dead current tile's
                f-plane (full width -> ordered after every sigma read), then
                staged to SBUF for the DMA and the next step's input."""
                j = t - (T - 1)
                nc.tensor.matmul(
                    cur_gts[0:OUT, 0, :],
                    lhsT=fc_wt_sb[:, :],
                    rhs=h_sb[:, :],
                    start=True,
                    stop=True,
                    skip_group_check=True,
                )
                nc.vector.tensor_copy(y_stage[0:OUT, :], cur_gts[0:OUT, 0, :])
                if j < FUT:
                    nc.sync.dma_start(out=y_out[:, j, :], in_=y_stage[0:OUT, :])

            def emit_step_forecast(t, cur_gts, nxt):
                """Pure forecast step (t >= T): this step's gates are built
                here — h-matmuls first (chunk 0 is the bank wiper via
                start=True), then the y-input projection accumulates with
                start=False. Avoids head-of-line blocking the PE queue on the
                y -> copy chain."""
                for ci, sl in enumerate(chunks):
                    h_matmuls_chunk(cur_gts, sl, start=(ci == 0))
                rhs, lhs = x_rhs(t)  # y_stage / forecast weights
                for g in range(4):
                    nc.tensor.matmul(
                        cur_gts[:, g, :],
                        lhsT=lhs[:, g, :],
                        rhs=rhs[:, :],
                        start=False,
                        stop=True,
                        skip_group_check=True,
                    )
                lag = VARIANT["tanh_lag"]
                for ci, sl in enumerate(chunks):
                    chunk_head(cur_gts, ci, sl)
                    if lag and ci >= 1:
                        chunk_tail2(ci - 1, chunks[ci - 1])
                    elif not lag:
                        chunk_tail2(ci, sl)
                if lag:
                    chunk_tail2(nchain - 1, chunks[nchain - 1])
                y_block(t, cur_gts)
                return nxt

            def emit_step(t, cur_gts):
                if t % xblk == 0:
                    fetch_x_block(t // xblk + xpf)

                nxt = None
                if t + 1 < nsteps or t >= T - 1:
                    nxt = psum.tile([H, 4, B], F32, name="gates", tag="g")
                if t >= T:
                    return emit_step_forecast(t, cur_gts, nxt)
                # PE queue: [h-matmuls(chunk0, t), x-matmuls(t+1), h-matmuls
                # (chunk1, t), ...] so the next-step x projection fills the
                # idle gap before the later chunks' h arrives. Forecast-phase
                # inputs (y) are produced by this step's tail, so those
                # matmuls move after the tails.
                # Emit each chunk's sigma before the next chunk's h-matmuls:
                # dependency edges follow program order, so a sigma emitted
                # after another chunk's h-matmuls on the shared tile would
                # pick up a spurious RAW wait on them.
                lag = VARIANT["tanh_lag"]
                for ci, sl in enumerate(chunks):
                    if t > 0:
                        h_matmuls_chunk(cur_gts, sl)
                    # Filler: redundant partial x-matmul keeps the PE busy
                    # through the inter-chunk idle window (pstate hold); the
                    # real start=True x-matmuls below overwrite it.
                    if VARIANT["x_filler"] and t + 1 < T and t + 1 < nsteps:
                        rhs_f, lhs_f = x_rhs(t + 1)
                        for g in range(VARIANT["x_filler"]):
                            nc.tensor.matmul(
                                nxt[:, g % 4, :],
                                lhsT=lhs_f[:, g % 4, :],
                                rhs=rhs_f[:, :],
                                start=True,
                                stop=False,
                                skip_group_check=True,
                            )
                    chunk_head(cur_gts, ci, sl)
                    if lag and ci >= 1:
                        chunk_tail2(ci - 1, chunks[ci - 1])
                    elif not lag:
                        chunk_tail2(ci, sl)
                if lag:
                    chunk_tail2(nchain - 1, chunks[nchain - 1])
                if t + 1 < T and t + 1 < nsteps:
                    x_matmuls_full(nxt, t + 1)

                if t == T - 1:
                    # First forecast output; the next step's gates are built
                    # inside emit_step_forecast (h-matmuls wipe, y accumulates).
                    y_block(t, cur_gts)
                return nxt

            # --- padded layout: per-chunk persistent tiles, one bank/gate ---
            BANKW = 512  # f32 elements per partition per PSUM bank

            def x_matmuls_padded(gtc, t, sl):
                rhs, lhs = x_rhs(t)
                for g in range(4):
                    nc.tensor.matmul(
                        gtc[:, g, 0 : sl.stop - sl.start],
                        lhsT=lhs[:, g, :],
                        rhs=rhs[:, sl],
                        start=True,
                        stop=(t == 0),
                        skip_group_check=True,
                    )

            def h_matmuls_padded(gtc, sl):
                for g in range(4):
                    nc.tensor.matmul(
                        gtc[:, g, 0 : sl.stop - sl.start],
                        lhsT=w_hh_sb[:, g, :],
                        rhs=h_sb[:, sl],
                        start=False,
                        stop=True,
                        skip_group_check=True,
                    )

            def emit_step_padded(t, gts):
                if t % xblk == 0:
                    fetch_x_block(t // xblk + xpf)
                for ci, sl in enumerate(chunks):
                    hb_c = sl.stop - sl.start
                    if t > 0:
                        h_matmuls_padded(gts[ci], sl)
                    chunk_tail(gts[ci], ci, sl, psl=slice(0, hb_c))
                    # Next-step x projection into the same banks; WAR on this
                    # chunk's sigma only, runs under the chunk's DVE tail.
                    if t + 1 < T and t + 1 < nsteps:
                        x_matmuls_padded(gts[ci], t + 1, sl)
                if t >= T - 1:
                    j = t - (T - 1)
                    for ci, sl in enumerate(chunks):
                        hb_c = sl.stop - sl.start
                        nc.tensor.matmul(
                            gts[ci][0:OUT, 0, 0:hb_c],
                            lhsT=fc_wt_sb[:, :],
                            rhs=h_sb[:, sl],
                            start=True,
                            stop=True,
                            skip_group_check=True,
                        )
                        nc.vector.tensor_copy(
                            y_stage[0:OUT, sl], gts[ci][0:OUT, 0, 0:hb_c]
                        )
                    if j < FUT:
                        nc.sync.dma_start(out=y_out[:, j, :], in_=y_stage[0:OUT, :])
                    if t + 1 < nsteps:
                        for ci, sl in enumerate(chunks):
                            x_matmuls_padded(gts[ci], t + 1, sl)

            def emit_steps():
                for bi in range(xpf):
                    fetch_x_block(bi)
                if VARIANT["layout"] == "padded":
                    gts = [
                        psum.tile([H, 4, BANKW], F32, name=f"gp{ci}", tag=f"gp{ci}")
                        for ci in range(nchain)
                    ]
                    for ci, sl in enumerate(chunks):
                        x_matmuls_padded(gts[ci], 0, sl)
                    for t in range(nsteps):
                        emit_step_padded(t, gts)
                    return
                gts = psum.tile([H, 4, B], F32, name="gates", tag="g")
                x_matmuls_full(gts, 0)
                for t in range(nsteps):
                    gts = emit_step(t, gts)

            if timing_reps > 1:
                with tc.For_i(0, timing_reps, 1):
                    emit_steps()
            else:
                emit_steps()
            if dump_state:
                nc.sync.dma_start(out=h_out[:, :], in_=h_sb[:, :])
                nc.sync.dma_start(out=c_out[:, :], in_=c_sb[:, :])
                nc.sync.dma_start(out=sig_out[:, :, :], in_=sig_sb[:, :, :])
                nc.sync.dma_start(out=t1_out[:, :], in_=t1_sb[:, :])
                nc.sync.dma_start(out=t2_out[:, :], in_=t2_sb[:, :])

    nc.compile()
    return nc


_NC_CACHE = None


def _get_nc():
    global _NC_CACHE
    if _NC_CACHE is None:
        _NC_CACHE = _build_nc()
    return _NC_CACHE


def _prep_weights(W_ih, W_hh, b_ih, b_hh, fc_w, fc_b):
    """Repack into gate order (f,i,o,g), g block pre-scaled by 2, biases
    folded; cast to fp16."""

    def blocks(mat):
        return {g: mat[_TORCH_SLOT[g] * H : (_TORCH_SLOT[g] + 1) * H] for g in _TORCH_SLOT}

    wih_b = blocks(W_ih)
    whh_b = blocks(W_hh)
    bias = b_ih + b_hh
    bias_b = blocks(bias)
    bias_fc_full = bias + W_ih @ fc_b
    bias_fc_b = blocks(bias_fc_full)

    w_hh_arr = np.empty((H, 4, H), np.float16)
    w_ih_e_arr = np.empty((3, 4, H), np.float16)
    w_ih_f_arr = np.empty((3, 4, H), np.float16)
    for gi, g in enumerate(_GATES):
        s = 2.0 if g == "g" else 1.0
        w_hh_arr[:, gi, :] = (s * whh_b[g].T).astype(np.float16)
        w_ih_e_arr[0:IN, gi, :] = (s * wih_b[g].T).astype(np.float16)
        w_ih_e_arr[2, gi, :] = (s * bias_b[g]).astype(np.float16)
        w_ih_f_arr[0:IN, gi, :] = (s * wih_b[g].T).astype(np.float16)
        w_ih_f_arr[2, gi, :] = (s * bias_fc_b[g]).astype(np.float16)
    fc_wt_arr = np.ascontiguousarray(fc_w.T).astype(np.float16)
    return w_hh_arr, w_ih_e_arr, w_ih_f_arr, fc_wt_arr


def kernel(x, W_ih, W_hh, b_ih, b_hh, fc_w, fc_b):
    x = np.asarray(x, np.float32)
    W_ih = np.asarray(W_ih, np.float32)
    W_hh = np.asarray(W_hh, np.float32)
    b_ih = np.asarray(b_ih, np.float32)
    b_hh = np.asarray(b_hh, np.float32)
    fc_w = np.asarray(fc_w, np.float32)
    fc_b = np.asarray(fc_b, np.float32)

    w_hh_arr, w_ih_e_arr, w_ih_f_arr, fc_wt_arr = _prep_weights(
        W_ih, W_hh, b_ih, b_hh, fc_w, fc_b
    )

    xblk = VARIANT["x_blk"]
    in_maps = []
    for k in range(NCORES):
        xs = x[k * B : (k + 1) * B]  # [B, T, IN]
        x_aug = np.empty((T // xblk, 3, xblk, B), np.float16)
        # [B, T, IN] -> [T, IN, B] -> [T//xblk, xblk, IN, B] -> transpose
        xt = xs.transpose(1, 2, 0).reshape(T // xblk, xblk, IN, B)
        x_aug[:, 0:IN, :, :] = xt.transpose(0, 2, 1, 3).astype(np.float16)
        x_aug[:, 2, :, :] = 1.0
        in_maps.append(
            {
                "x_aug": np.ascontiguousarray(x_aug),
                "w_hh": w_hh_arr,
                "w_ih_e": w_ih_e_arr,
                "w_ih_f": w_ih_f_arr,
                "fc_wt": fc_wt_arr,
                "ones3": np.ones((3, B), np.float16),
            }
        )

    nc = _get_nc()
    res = run_bass_kernel_spmd(nc, in_maps, core_ids=list(range(NCORES)))

    out = np.empty((B_TOT, FUT * OUT), np.float32)
    bias_tile = np.tile(fc_b, FUT).astype(np.float32)
    for k in range(NCORES):
        ys = res.results[k]["y_out"].astype(np.float32)  # [OUT, FUT, B]
        out[k * B : (k + 1) * B] = ys.transpose(2, 1, 0).reshape(B, FUT * OUT)
    out += bias_tile
    return out

